# revision 21
# baseline (speedup 1.0000x reference)
"""Trainium2 Bass kernel for nn_CumulativeFlattenedLinear (segment_reduce).

Computation: per window of S=64 timesteps, per-timestep C->O linear projection
(weights zero for the first n_discard steps) followed by a causal cumsum within
the window, plus bias.

Strategy (data-parallel over batch, 1 batch element per core):
  - x loaded f32 via HWDGE (sync queue), partition = 256-element time chunk,
    1KB contiguous DRAM runs; supertile 0 split in halves to start earlier.
  - Per window: DVE gathers the active sub-blocks into (u, c, v) column order,
    casting f32->bf16; per 8-step sub-block u the 128x128 block is transposed
    on the TensorEngine (bf16: 1 cyc/row), then one bf16 matmul per sub-block
    against a host-built triangular weight block (columns ordered (o, v))
    computes the intra-block causal cumsum of projections: pw[p, (o,v)] PSUM.
  - Cross-sub-block prefix: ACT copies the five block totals (v=7 lanes) to
    SBUF, GpSimd chains them with the bias seed; one strided DVE add per
    window combines intra + prefix into the (o, t)-ordered bf16 output tile;
    ACT fills the discarded head with bias.
  - y is stored as bf16 (rel err ~4e-3 << 2e-2 gate) and upcast to f32 on
    host; HBM traffic is 12MB/core (8 in + 4 out) ~= 34us at 358 GB/s.
"""
import numpy as np

import concourse.bass as bass
import concourse.tile as tile
from concourse import bacc, mybir
from concourse.bass_utils import run_bass_kernel_spmd

F32 = mybir.dt.float32
BF16 = mybir.dt.bfloat16

# problem geometry (asserted against inputs at runtime)
B, C, T, O = 8, 16, 131072, 16
P = 128
CH = 256                 # time-elements per partition per supertile
NST = T // (P * CH)      # 4 supertiles
V = 8                    # sub-block length
NU = 8                   # sub-blocks per window

_cache = {}


def _build_nc(du_count):
    """Build the per-core Bass program. du_count = number of active sub-blocks
    (those with any nonzero weight), assumed to be the trailing ones."""
    S = NU * V  # 64
    NW = CH // S  # windows per partition = 4
    DU = du_count
    first_u = NU - DU          # first active sub-block
    fill_s = first_u * V       # s < fill_s -> output = bias

    nc = bacc.Bacc("TRN2", target_bir_lowering=False, debug=False)
    x_d = nc.dram_tensor("x", (C, T), F32, kind="ExternalInput")
    w_d = nc.dram_tensor("w_all", (P, DU * 128), BF16, kind="ExternalInput")
    b16_d = nc.dram_tensor("bias16", (P, O), F32, kind="ExternalInput")
    ident_d = nc.dram_tensor("ident", (P, P), BF16, kind="ExternalInput")
    bfill_d = nc.dram_tensor("bias_fill", (P, O * fill_s), BF16,
                             kind="ExternalInput")
    y_d = nc.dram_tensor("y", (O, T), BF16, kind="ExternalOutput")

    xv = x_d.ap().rearrange("c (st p hs) -> st p c hs", st=NST, p=P, hs=CH)
    yv = y_d.ap().rearrange("o (st p hs) -> st p o hs", st=NST, p=P, hs=CH)

    with tile.TileContext(nc) as tc:
        SKEW = 3
        with (
            tc.tile_pool(name="const", bufs=1) as cp,
            tc.tile_pool(name="io", bufs=3) as io,
            tc.tile_pool(name="mid", bufs=SKEW + 2) as mid,
            tc.tile_pool(name="psT", bufs=2, space="PSUM") as psT,
            tc.tile_pool(name="psW", bufs=3, space="PSUM") as psW,
        ):
            xins = {}
            CQ = 4          # channels per load chunk: 4 DMAs per supertile

            def prefetch(st, parts):
                if st not in xins:
                    xins[st] = io.tile([P, C * CH], F32, name="xin", tag="xin")
                xr = xins[st][:].rearrange("p (c hs) -> p c hs", c=C)
                for q in parts:
                    nc.sync.dma_start(
                        xr[:, q * CQ:(q + 1) * CQ],
                        xv[st][:, q * CQ:(q + 1) * CQ],
                    )

            # first supertile load chunks go out before the consts
            prefetch(0, range(C // CQ))

            w_all = cp.tile([P, DU * 128], BF16, name="w_all")
            nc.scalar.dma_start(w_all[:], w_d.ap())
            bias16 = cp.tile([P, O], F32, name="bias16")
            nc.scalar.dma_start(bias16[:], b16_d.ap())
            ident = cp.tile([P, P], BF16, name="ident")
            nc.scalar.dma_start(ident[:], ident_d.ap())
            bfill = cp.tile([P, O * fill_s], BF16, name="bfill")
            nc.scalar.dma_start(bfill[:], bfill_d.ap())
            # statically-seeded prefix tiles (col 0:O = bias, never rewritten)
            pre_tiles = []
            for k in range(4):
                pre = cp.tile([P, DU * O], F32, name=f"pre{k}")
                nc.vector.tensor_copy(pre[:, 0:O], bias16[:])
                pre_tiles.append(pre)

            if NST > 1:
                prefetch(1, range(C // CQ))

            pending = []

            for st in range(NST):
                xin = xins.pop(st)
                out_sb = io.tile([P, O * CH], BF16, name="out_sb", tag="out")
                for wdw in range(NW):
                    # ---- stage A: shuffle (DVE, f32->bf16 cast)
                    #      + transposes (PE) + PSUM->SBUF copy (ACT)
                    shuf = mid.tile([P, DU * 128], BF16, name="shuf",
                                    tag="shuf")
                    src = xin[:].rearrange(
                        "p (c w u v) -> w p u c v", c=C, w=NW, u=NU, v=V
                    )[wdw, :, first_u:NU]
                    nc.vector.tensor_copy(
                        shuf[:].rearrange("p (u c v) -> p u c v",
                                          u=DU, c=C, v=V),
                        src,
                    )
                    pt = psT.tile([P, DU * 128], BF16, name="pt", tag="pt")
                    for i in range(DU):
                        nc.tensor.transpose(
                            pt[:, i * 128:(i + 1) * 128],
                            shuf[:, i * 128:(i + 1) * 128],
                            ident[:],
                            tile_position=(0, 0),
                        )
                    ts = mid.tile([P, DU * 128], BF16, name="ts", tag="ts")
                    nc.scalar.copy(ts[:], pt[:])

                    if len(pending) >= SKEW:
                        pending.pop(0)()

                    def stage_b(st=st, wdw=wdw, ts=ts, out_sb=out_sb):
                        # ---- matmuls: intra-block triangular projections
                        #      pw columns ordered (u, o, v)
                        pw = psW.tile([P, DU * 128], F32, name="pw", tag="pw")
                        for i in range(DU):
                            nc.tensor.matmul(
                                pw[:, i * 128:(i + 1) * 128],
                                ts[:, i * 128:(i + 1) * 128],
                                w_all[:, i * 128:(i + 1) * 128],
                                start=True,
                                stop=True,
                            )
                        # ---- block totals (v = V-1 lanes) -> SBUF (ACT)
                        tot = mid.tile([P, (DU - 1) * O], F32, name="tot",
                                       tag="tot")
                        nc.scalar.copy(
                            tot[:].rearrange("p (u o) -> p u o", u=DU - 1),
                            pw[:].rearrange(
                                "p (u o v) -> p u o v", u=DU, o=O, v=V
                            )[:, 0:DU - 1, :, V - 1],
                        )
                        # ---- prefix chain with bias seed (GpSimd)
                        pre = pre_tiles[(st * NW + wdw) % 4]
                        for i in range(1, DU):
                            nc.gpsimd.tensor_add(
                                pre[:, i * O:(i + 1) * O],
                                pre[:, (i - 1) * O:i * O],
                                tot[:, (i - 1) * O:i * O],
                            )
                        # ---- combine: out[(o, s)] = intra + pre_bcast (DVE)
                        out4 = out_sb[:].rearrange(
                            "p (o w u v) -> w p o u v", o=O, w=NW, u=NU, v=V
                        )[wdw, :, :, first_u:NU]
                        in1 = pw[:].rearrange(
                            "p (u o v) -> p o u v", u=DU, o=O, v=V
                        )
                        in2 = pre[:].rearrange("p (u o) -> p o u", u=DU)
                        in2 = in2.unsqueeze(3).broadcast_to([P, O, DU, V])
                        nc.vector.tensor_add(out4, in1, in2)
                        if wdw == 0:
                            # ---- bias fill for s < fill_s, all windows (ACT)
                            outf = out_sb[:].rearrange(
                                "p (o w s) -> p o w s", o=O, w=NW
                            )[:, :, :, 0:fill_s]
                            nc.scalar.copy(
                                outf,
                                bfill[:].rearrange("p (o s) -> p o s", o=O)
                                .unsqueeze(2).broadcast_to([P, O, NW, fill_s]),
                            )
                        if wdw == NW - 1:
                            nc.scalar.dma_start(
                                yv[st],
                                out_sb[:].rearrange("p (o hs) -> p o hs", o=O),
                            )

                    pending.append(stage_b)
                # prefetch after this supertile's compute is emitted, so
                # earlier tiles' readers don't wait on these loads'
                # semaphore-lane counts
                if st + 2 < NST:
                    prefetch(st + 2, range(C // CQ))
            for fn in pending:
                fn()
    nc.compile()
    return nc


def _host_constants(weight, bias, n_discard, n_keep):
    S = n_discard + n_keep
    assert S == NU * V
    w = weight.reshape(O, C, n_keep).transpose(2, 1, 0)  # (n_keep, C, O)
    w_full = np.concatenate(
        [np.zeros((n_discard, C, O), np.float32), w.astype(np.float32)], axis=0
    )  # (S, C, O)
    act = [u for u in range(NU)
           if np.abs(w_full[u * V:(u + 1) * V]).max() > 0]
    # kernel assumes active blocks are trailing & contiguous
    first_u = act[0] if act else NU
    assert act == list(range(first_u, NU))
    DU = len(act)
    fill_s = first_u * V
    bf16 = mybir.dt.np(BF16)
    w_all = np.zeros((P, DU * 128), np.float32)
    for idx, u in enumerate(act):
        blk = w_full[u * V:(u + 1) * V]  # (V, C, O)
        # Wtri: k=(c,vp) -> n=(o,v), vp <= v
        tri = np.zeros((C, V, O, V), np.float32)
        for v in range(V):
            for vp in range(v + 1):
                tri[:, vp, :, v] = blk[vp]
        w_all[:, idx * 128:(idx + 1) * 128] = tri.reshape(C * V, O * V)
    bias32 = bias.astype(np.float32)
    consts = {
        "w_all": np.ascontiguousarray(w_all).astype(bf16),
        "bias16": np.ascontiguousarray(
            bias32[None, :] * np.ones((P, 1), np.float32)
        ),
        "ident": np.eye(P, dtype=np.float32).astype(bf16),
        "bias_fill": np.ascontiguousarray(
            np.tile(bias32[:, None], (1, fill_s)).reshape(1, -1)
            * np.ones((P, 1), np.float32)
        ).astype(bf16),
    }
    return consts, DU


def _run(inputs, trace=False):
    x = np.asarray(inputs["x"], dtype=np.float32)
    weight = np.asarray(inputs["weight"], dtype=np.float32)
    bias = np.asarray(inputs["bias"], dtype=np.float32)
    n_discard = int(inputs["n_discard"])
    n_keep = int(inputs["n_keep"])
    assert x.shape == (B, C, T) and weight.shape == (O, C * n_keep)

    consts, DU = _host_constants(weight, bias, n_discard, n_keep)
    key = ("nc", DU)
    if key not in _cache:
        _cache[key] = _build_nc(DU)
    nc = _cache[key]

    in_maps = []
    for b in range(B):
        m = dict(consts)
        m["x"] = np.ascontiguousarray(x[b])
        in_maps.append(m)
    res = run_bass_kernel_spmd(nc, in_maps, list(range(B)), trace=trace)
    y = np.stack(
        [res.results[b]["y"].astype(np.float32) for b in range(B)], axis=0
    )
    return y, res


def kernel(**inputs):
    y, _ = _run(inputs, trace=False)
    return y


# revision 24
# speedup vs baseline: 1.0210x; 1.0210x over previous
"""Trainium2 Bass kernel for nn_CumulativeFlattenedLinear (segment_reduce).

Computation: per window of S=64 timesteps, per-timestep C->O linear projection
(weights zero for the first n_discard steps) followed by a causal cumsum within
the window, plus bias.

Strategy (data-parallel over batch, 1 batch element per core):
  - x loaded f32 via HWDGE (sync queue), partition = 256-element time chunk,
    1KB contiguous DRAM runs; supertile 0 split in halves to start earlier.
  - Per window: DVE gathers the active sub-blocks into (u, c, v) column order,
    casting f32->bf16; per 8-step sub-block u the 128x128 block is transposed
    on the TensorEngine (bf16: 1 cyc/row), then one bf16 matmul per sub-block
    against a host-built triangular weight block (columns ordered (o, v))
    computes the intra-block causal cumsum of projections: pw[p, (o,v)] PSUM.
  - Cross-sub-block prefix: ACT copies the five block totals (v=7 lanes) to
    SBUF, GpSimd chains them with the bias seed; one strided DVE add per
    window combines intra + prefix into the (o, t)-ordered bf16 output tile;
    ACT fills the discarded head with bias.
  - y is stored as bf16 (rel err ~4e-3 << 2e-2 gate) and upcast to f32 on
    host; HBM traffic is 12MB/core (8 in + 4 out) ~= 34us at 358 GB/s.
"""
import numpy as np

import concourse.bass as bass
import concourse.tile as tile
from concourse import bacc, mybir
from concourse.bass_utils import run_bass_kernel_spmd

F32 = mybir.dt.float32
BF16 = mybir.dt.bfloat16

# problem geometry (asserted against inputs at runtime)
B, C, T, O = 8, 16, 131072, 16
P = 128
CH = 256                 # time-elements per partition per supertile
NST = T // (P * CH)      # 4 supertiles
V = 8                    # sub-block length
NU = 8                   # sub-blocks per window

_cache = {}


def _build_nc(du_count):
    """Build the per-core Bass program. du_count = number of active sub-blocks
    (those with any nonzero weight), assumed to be the trailing ones."""
    S = NU * V  # 64
    NW = CH // S  # windows per partition = 4
    DU = du_count
    first_u = NU - DU          # first active sub-block
    fill_s = first_u * V       # s < fill_s -> output = bias

    nc = bacc.Bacc("TRN2", target_bir_lowering=False, debug=False)
    x_d = nc.dram_tensor("x", (C, T), F32, kind="ExternalInput")
    w_d = nc.dram_tensor("w_all", (P, DU * 128), BF16, kind="ExternalInput")
    b16_d = nc.dram_tensor("bias16", (P, O), F32, kind="ExternalInput")
    ident_d = nc.dram_tensor("ident", (P, P), BF16, kind="ExternalInput")
    bfill_d = nc.dram_tensor("bias_fill", (P, O * fill_s), BF16,
                             kind="ExternalInput")
    y_d = nc.dram_tensor("y", (O, T), BF16, kind="ExternalOutput")

    xv = x_d.ap().rearrange("c (st p hs) -> st p c hs", st=NST, p=P, hs=CH)
    yv = y_d.ap().rearrange("o (st p hs) -> st p o hs", st=NST, p=P, hs=CH)

    with tile.TileContext(nc) as tc:
        SKEW = 3
        with (
            tc.tile_pool(name="const", bufs=1) as cp,
            tc.tile_pool(name="io", bufs=3) as io,
            tc.tile_pool(name="mid", bufs=SKEW + 2) as mid,
            tc.tile_pool(name="psT", bufs=2, space="PSUM") as psT,
            tc.tile_pool(name="psW", bufs=3, space="PSUM") as psW,
        ):
            xins = {}
            CQ = 4          # channels per load chunk: 4 DMAs per supertile

            def prefetch(st, parts):
                if st not in xins:
                    xins[st] = io.tile([P, C * CH], F32, name="xin", tag="xin")
                xr = xins[st][:].rearrange("p (c hs) -> p c hs", c=C)
                for q in parts:
                    nc.sync.dma_start(
                        xr[:, q * CQ:(q + 1) * CQ],
                        xv[st][:, q * CQ:(q + 1) * CQ],
                    )

            # first supertile load chunks go out before the consts.
            # Only ONE supertile load streams at a time: concurrent DMAs
            # share SDMA bandwidth round-robin, so overlapping two tile
            # loads delays BOTH tiles' completion semaphores.
            prefetch(0, range(C // CQ))

            w_all = cp.tile([P, DU * 128], BF16, name="w_all")
            nc.scalar.dma_start(w_all[:], w_d.ap())
            bias16 = cp.tile([P, O], F32, name="bias16")
            nc.scalar.dma_start(bias16[:], b16_d.ap())
            ident = cp.tile([P, P], BF16, name="ident")
            nc.scalar.dma_start(ident[:], ident_d.ap())
            bfill = cp.tile([P, O * fill_s], BF16, name="bfill")
            nc.scalar.dma_start(bfill[:], bfill_d.ap())
            # statically-seeded prefix tiles (col 0:O = bias, never rewritten)
            pre_tiles = []
            for k in range(4):
                pre = cp.tile([P, DU * O], F32, name=f"pre{k}")
                nc.vector.tensor_copy(pre[:, 0:O], bias16[:])
                pre_tiles.append(pre)

            pending = []

            for st in range(NST):
                if st + 1 < NST:
                    prefetch(st + 1, range(C // CQ))
                xin = xins.pop(st)
                out_sb = io.tile([P, O * CH], BF16, name="out_sb", tag="out")
                for wdw in range(NW):
                    # ---- stage A: shuffle (DVE, f32->bf16 cast)
                    #      + transposes (PE) + PSUM->SBUF copy (ACT)
                    shuf = mid.tile([P, DU * 128], BF16, name="shuf",
                                    tag="shuf")
                    src = xin[:].rearrange(
                        "p (c w u v) -> w p u c v", c=C, w=NW, u=NU, v=V
                    )[wdw, :, first_u:NU]
                    nc.vector.tensor_copy(
                        shuf[:].rearrange("p (u c v) -> p u c v",
                                          u=DU, c=C, v=V),
                        src,
                    )
                    pt = psT.tile([P, DU * 128], BF16, name="pt", tag="pt")
                    for i in range(DU):
                        nc.tensor.transpose(
                            pt[:, i * 128:(i + 1) * 128],
                            shuf[:, i * 128:(i + 1) * 128],
                            ident[:],
                            tile_position=(0, 0),
                        )
                    ts = mid.tile([P, DU * 128], BF16, name="ts", tag="ts")
                    nc.scalar.copy(ts[:], pt[:])

                    if len(pending) >= SKEW:
                        pending.pop(0)()

                    def stage_b(st=st, wdw=wdw, ts=ts, out_sb=out_sb):
                        # ---- matmuls: intra-block triangular projections
                        #      pw columns ordered (u, o, v)
                        pw = psW.tile([P, DU * 128], F32, name="pw", tag="pw")
                        for i in range(DU):
                            nc.tensor.matmul(
                                pw[:, i * 128:(i + 1) * 128],
                                ts[:, i * 128:(i + 1) * 128],
                                w_all[:, i * 128:(i + 1) * 128],
                                start=True,
                                stop=True,
                            )
                        # ---- block totals (v = V-1 lanes) -> SBUF (ACT)
                        tot = mid.tile([P, (DU - 1) * O], F32, name="tot",
                                       tag="tot")
                        nc.scalar.copy(
                            tot[:].rearrange("p (u o) -> p u o", u=DU - 1),
                            pw[:].rearrange(
                                "p (u o v) -> p u o v", u=DU, o=O, v=V
                            )[:, 0:DU - 1, :, V - 1],
                        )
                        # ---- prefix chain with bias seed (GpSimd)
                        pre = pre_tiles[(st * NW + wdw) % 4]
                        for i in range(1, DU):
                            nc.gpsimd.tensor_add(
                                pre[:, i * O:(i + 1) * O],
                                pre[:, (i - 1) * O:i * O],
                                tot[:, (i - 1) * O:i * O],
                            )
                        # ---- combine: out[(o, s)] = intra + pre_bcast (DVE)
                        out4 = out_sb[:].rearrange(
                            "p (o w u v) -> w p o u v", o=O, w=NW, u=NU, v=V
                        )[wdw, :, :, first_u:NU]
                        in1 = pw[:].rearrange(
                            "p (u o v) -> p o u v", u=DU, o=O, v=V
                        )
                        in2 = pre[:].rearrange("p (u o) -> p o u", u=DU)
                        in2 = in2.unsqueeze(3).broadcast_to([P, O, DU, V])
                        nc.vector.tensor_add(out4, in1, in2)
                        if wdw == 0:
                            # ---- bias fill for s < fill_s, all windows (ACT)
                            outf = out_sb[:].rearrange(
                                "p (o w s) -> p o w s", o=O, w=NW
                            )[:, :, :, 0:fill_s]
                            nc.scalar.copy(
                                outf,
                                bfill[:].rearrange("p (o s) -> p o s", o=O)
                                .unsqueeze(2).broadcast_to([P, O, NW, fill_s]),
                            )
                        if wdw == NW - 1:
                            nc.scalar.dma_start(
                                yv[st],
                                out_sb[:].rearrange("p (o hs) -> p o hs", o=O),
                            )

                    pending.append(stage_b)
            for fn in pending:
                fn()
    nc.compile()
    return nc


def _host_constants(weight, bias, n_discard, n_keep):
    S = n_discard + n_keep
    assert S == NU * V
    w = weight.reshape(O, C, n_keep).transpose(2, 1, 0)  # (n_keep, C, O)
    w_full = np.concatenate(
        [np.zeros((n_discard, C, O), np.float32), w.astype(np.float32)], axis=0
    )  # (S, C, O)
    act = [u for u in range(NU)
           if np.abs(w_full[u * V:(u + 1) * V]).max() > 0]
    # kernel assumes active blocks are trailing & contiguous
    first_u = act[0] if act else NU
    assert act == list(range(first_u, NU))
    DU = len(act)
    fill_s = first_u * V
    bf16 = mybir.dt.np(BF16)
    w_all = np.zeros((P, DU * 128), np.float32)
    for idx, u in enumerate(act):
        blk = w_full[u * V:(u + 1) * V]  # (V, C, O)
        # Wtri: k=(c,vp) -> n=(o,v), vp <= v
        tri = np.zeros((C, V, O, V), np.float32)
        for v in range(V):
            for vp in range(v + 1):
                tri[:, vp, :, v] = blk[vp]
        w_all[:, idx * 128:(idx + 1) * 128] = tri.reshape(C * V, O * V)
    bias32 = bias.astype(np.float32)
    consts = {
        "w_all": np.ascontiguousarray(w_all).astype(bf16),
        "bias16": np.ascontiguousarray(
            bias32[None, :] * np.ones((P, 1), np.float32)
        ),
        "ident": np.eye(P, dtype=np.float32).astype(bf16),
        "bias_fill": np.ascontiguousarray(
            np.tile(bias32[:, None], (1, fill_s)).reshape(1, -1)
            * np.ones((P, 1), np.float32)
        ).astype(bf16),
    }
    return consts, DU


def _run(inputs, trace=False):
    x = np.asarray(inputs["x"], dtype=np.float32)
    weight = np.asarray(inputs["weight"], dtype=np.float32)
    bias = np.asarray(inputs["bias"], dtype=np.float32)
    n_discard = int(inputs["n_discard"])
    n_keep = int(inputs["n_keep"])
    assert x.shape == (B, C, T) and weight.shape == (O, C * n_keep)

    consts, DU = _host_constants(weight, bias, n_discard, n_keep)
    key = ("nc", DU)
    if key not in _cache:
        _cache[key] = _build_nc(DU)
    nc = _cache[key]

    in_maps = []
    for b in range(B):
        m = dict(consts)
        m["x"] = np.ascontiguousarray(x[b])
        in_maps.append(m)
    res = run_bass_kernel_spmd(nc, in_maps, list(range(B)), trace=trace)
    y = np.stack(
        [res.results[b]["y"].astype(np.float32) for b in range(B)], axis=0
    )
    return y, res


def kernel(**inputs):
    y, _ = _run(inputs, trace=False)
    return y
